# revision 12
# baseline (speedup 1.0000x reference)
"""Trainium2 Bass kernel for nn_BoilerplateLoss (softmax-margin + generalized-mean loss).

Reference computation per row (B=32768 rows, C=1000 classes, K=10 attack idx):
    probs = softmax(y_pred)
    in_att = probs[y_attack]                       # [K]
    macro  = max(probs outside attack) - min(in_att)
    s      = 5 + 5*diff(in_att)                    # [K-1]
    gm9    = mean(s^9)^(1/9)
    sorting = (gm9 - 5)/5
    out    = (mean([(5+5*macro)^10, (5+5*sorting)^10])^(1/10) - 5)/5

Device/host split: the only O(B*C) work is the softmax denominator
Z = sum(exp(x)) and the complement max (max logit outside the attack set).
Everything else is O(B*K) and runs on the host in float64.

The attack mask is applied HOST-side: y_pred is cast to fp16 and the 10
attacked positions per row are overwritten with -1000 before upload, so the
device needs no scatter/mask machinery (exp(-1000)=0 drops the attacked
cols from Z; max skips them since -1000 < any real logit). The host adds
back sum(exp(attack logits)) to reconstruct the full Z.

Per-core device pipeline (4096 rows = 8 supertiles of [128 part x 4 rows x
1000 cols] fp16; 4 consecutive DRAM rows per partition = 8 KB contiguous
DMA descriptors):
  - max: TENSOR_TENSOR max fold tree at DVE 2x rate (1000->500->250->125
    cols, all-16-bit operands hit the 2x perf mode; TENSOR_REDUCE is
    always 1x on this runtime) + one final 1x TENSOR_REDUCE of 125 cols.
  - Z: split to balance engines. For ACT_Z supertiles the ACT engine
    accumulates the row sum during the exp pass (per-group instructions,
    output to a broadcast dummy); for the rest, ACT does one wide pure exp
    and the DVE sums E with the same 2x fold tree + 1x TR(add, f32 out).
fp16 (not bf16): logits are N(0,1) so range is irrelevant and the 3 extra
mantissa bits put input-quantization error at ~7e-4 norm-rel.
"""

import numpy as np

import concourse.bacc as bacc
import concourse.bass as bass  # noqa: F401
import concourse.mybir as mybir
import concourse.tile as tile
from concourse.bass_utils import run_bass_kernel_spmd

B, C, K = 32768, 1000, 10
N_CORES = 8
ROWS = B // N_CORES  # 4096 rows per core
P = 128  # SBUF partitions
R = 8  # consecutive DRAM rows packed per partition
NS = ROWS // (P * R)  # 8 supertiles per core
NT = NS * R  # 32 output columns per partition
N_ACT_Z = 2  # last N supertiles accumulate Z on the ACT engine
MASK_VAL = np.float16(-1000.0)
CC = 5.0

f16 = mybir.dt.float16
f32 = mybir.dt.float32

_CACHE = {}


def build_nc(rows=ROWS):
    """Build the Bass program for one core's shard of `rows` rows."""
    assert rows == NS * P * R

    nc = bacc.Bacc("TRN2", target_bir_lowering=False, debug=False)

    yp = nc.dram_tensor("yp", [rows, C], f16, kind="ExternalInput").ap()
    mx_out = nc.dram_tensor("mx", [P, NT], f16, kind="ExternalOutput").ap()
    zs_out = nc.dram_tensor("zs", [P, NT], f32, kind="ExternalOutput").ap()

    # supertile u, partition p holds DRAM rows u*512 + p*4 + [0..3]
    ypt = yp.rearrange("(u p r) c -> u p (r c)", p=P, r=R)

    Alu = mybir.AluOpType
    Act = mybir.ActivationFunctionType
    X = mybir.AxisListType.X

    with tile.TileContext(nc) as tc:
        with (
            tc.tile_pool(name="singles", bufs=1) as singles,
            tc.tile_pool(name="lg", bufs=4) as lgp,
            tc.tile_pool(name="ep", bufs=3) as epp,
            tc.tile_pool(name="scr", bufs=3) as scrp,
        ):
            MX = singles.tile([P, NT], f16)
            ZS = singles.tile([P, NT], f32)

            def fold_tree(src3, rr, op, tag):
                """[P, rr, 1000] -> [P, rr, 125] via three 2x TT folds."""
                f1 = scrp.tile([P, R * 500], f16, tag=tag + "1")
                f13 = f1[:, 0 : rr * 500].rearrange("p (r c) -> p r c", c=500)
                nc.vector.tensor_tensor(
                    out=f13, in0=src3[:, :, 0:500], in1=src3[:, :, 500:1000], op=op
                )
                f2 = scrp.tile([P, R * 250], f16, tag=tag + "2")
                f23 = f2[:, 0 : rr * 250].rearrange("p (r c) -> p r c", c=250)
                nc.vector.tensor_tensor(
                    out=f23, in0=f13[:, :, 0:250], in1=f13[:, :, 250:500], op=op
                )
                f3 = scrp.tile([P, R * 125], f16, tag=tag + "3")
                f33 = f3[:, 0 : rr * 125].rearrange("p (r c) -> p r c", c=125)
                nc.vector.tensor_tensor(
                    out=f33, in0=f23[:, :, 0:125], in1=f23[:, :, 125:250], op=op
                )
                return f33

            def emit_compute(lg, u, r0, rr, act_z):
                """Process rows [r0, r0+rr) of supertile u (tile `lg`)."""
                lg3 = lg[:, r0 * C : (r0 + rr) * C].rearrange(
                    "p (r c) -> p r c", c=C
                )
                t0 = u * R + r0
                # complement max of raw logits: 2x fold tree + 1x final TR
                m33 = fold_tree(lg3, rr, Alu.max, "m")
                nc.vector.tensor_reduce(
                    out=MX[:, t0 : t0 + rr], in_=m33, axis=X, op=Alu.max
                )
                if act_z:
                    # Z on the ACT accumulator (per-group exp, dummy output)
                    for r in range(rr):
                        dummy = scrp.tile([P, 1], f32, tag="dm")
                        nc.scalar.activation(
                            out=dummy[:].broadcast_to([P, C]),
                            in_=lg3[:, r, :],
                            func=Act.Exp,
                            accum_out=ZS[:, t0 + r : t0 + r + 1],
                        )
                else:
                    # Z on DVE: wide pure exp, then 2x fold tree + 1x TR add
                    E = epp.tile([P, R * C], f16)
                    nc.scalar.activation(
                        out=E[:, 0 : rr * C], in_=lg[:, r0 * C : (r0 + rr) * C],
                        func=Act.Exp,
                    )
                    E3 = E[:, 0 : rr * C].rearrange("p (r c) -> p r c", c=C)
                    s33 = fold_tree(E3, rr, Alu.add, "s")
                    nc.vector.tensor_reduce(
                        out=ZS[:, t0 : t0 + rr], in_=s33, axis=X, op=Alu.add
                    )

            # first supertiles arrive in row-chunks so compute starts as
            # soon as the first 256 KB lands and isn't gated on the DMA ramp
            CHUNKS = {0: (1, 1, 2, 4), 1: (4, 4)}
            for u in range(NS):
                lg = lgp.tile([P, R * C], f16)
                act_z = u >= NS - N_ACT_Z
                r0 = 0
                for rr in CHUNKS.get(u, (R,)):
                    nc.sync.dma_start(
                        out=lg[:, r0 * C : (r0 + rr) * C],
                        in_=ypt[u, :, r0 * C : (r0 + rr) * C],
                    )
                    emit_compute(lg, u, r0, rr, act_z)
                    r0 += rr

            # bulk of mx/zs is ready after supertile 6; issue those on the
            # (now idle) Sync queue, and the last supertile's slices from the
            # Vector/Scalar queues so each trigger fires right after its
            # producer without serializing on one queue.
            tl = (NS - 1) * R
            nc.sync.dma_start(out=mx_out[:, 0:tl], in_=MX[:, 0:tl])
            nc.sync.dma_start(out=zs_out[:, 0:tl], in_=ZS[:, 0:tl])
            nc.sync.dma_start(out=mx_out[:, tl:NT], in_=MX[:, tl:NT])
            nc.scalar.dma_start(out=zs_out[:, tl:NT], in_=ZS[:, tl:NT])

    nc.compile()
    return nc


def _uncol(a):
    """[P, NT] per-core output -> [ROWS] vector; row u*512 + p*4 + r."""
    return np.ascontiguousarray(
        a.reshape(P, NS, R).transpose(1, 0, 2).reshape(ROWS)
    )


def kernel(y_pred, y_attack, _trace=False, _trace_kwargs=None):
    """Full-input entry point: shards across 8 NeuronCores, returns [B] f32."""
    y_pred = np.asarray(y_pred, dtype=np.float32)
    y_attack = np.asarray(y_attack, dtype=np.int32)
    assert y_pred.shape == (B, C) and y_attack.shape == (B, K)

    if "nc" not in _CACHE:
        _CACHE["nc"] = build_nc(ROWS)
    nc = _CACHE["nc"]

    ya = y_attack.astype(np.int64)
    rows_idx = np.arange(B)[:, None]
    yh = y_pred.astype(np.float16)
    attl = np.take_along_axis(yh, ya, axis=1).astype(np.float64)  # [B, K]
    yh[rows_idx, ya] = MASK_VAL

    in_maps = [
        {"yp": np.ascontiguousarray(yh[c * ROWS : (c + 1) * ROWS])}
        for c in range(N_CORES)
    ]
    kwargs = dict(_trace_kwargs or {})
    res = run_bass_kernel_spmd(
        nc, in_maps, core_ids=list(range(N_CORES)), trace=_trace, **kwargs
    )

    MXv = np.empty((B,), dtype=np.float64)
    ZC = np.empty((B,), dtype=np.float64)
    for c in range(N_CORES):
        s = slice(c * ROWS, (c + 1) * ROWS)
        MXv[s] = _uncol(np.asarray(res.results[c]["mx"], dtype=np.float64))
        ZC[s] = _uncol(np.asarray(res.results[c]["zs"], dtype=np.float64))

    # Host epilogue in f64 (O(B*K) work), mirroring reference's f32 casts
    # after each generalized mean.
    att_exp = np.exp(attl)  # [B, K]
    Z = ZC + att_exp.sum(axis=1)
    p_att = att_exp / Z[:, None]
    cmax = np.exp(MXv) / Z
    macro = cmax - p_att.min(axis=1)
    s9 = CC + CC * np.diff(p_att, axis=1)
    gm9 = (np.mean(s9**9.0, axis=1)) ** (1.0 / 9.0)
    gm9 = gm9.astype(np.float32).astype(np.float64)
    sorting = (gm9 - CC) / CC
    catted = np.stack([macro, sorting], axis=-1)
    gm10 = (np.mean((CC + CC * catted) ** 10.0, axis=1)) ** 0.1
    y = ((gm10.astype(np.float32) - CC) / CC).astype(np.float32)

    if _trace:
        return y, res
    return y


# revision 13
# speedup vs baseline: 1.0895x; 1.0895x over previous
"""Trainium2 Bass kernel for nn_BoilerplateLoss (softmax-margin + generalized-mean loss).

Reference computation per row (B=32768 rows, C=1000 classes, K=10 attack idx):
    probs = softmax(y_pred)
    in_att = probs[y_attack]                       # [K]
    macro  = max(probs outside attack) - min(in_att)
    s      = 5 + 5*diff(in_att)                    # [K-1]
    gm9    = mean(s^9)^(1/9)
    sorting = (gm9 - 5)/5
    out    = (mean([(5+5*macro)^10, (5+5*sorting)^10])^(1/10) - 5)/5

Device/host split: the only O(B*C) work is the softmax denominator
Z = sum(exp(x)) and the complement max (max logit outside the attack set).
Everything else is O(B*K) and runs on the host in float64.

The attack mask is applied HOST-side: y_pred is cast to fp16 and the 10
attacked positions per row are overwritten with -1000 before upload, so the
device needs no scatter/mask machinery (exp(-1000)=0 drops the attacked
cols from Z; max skips them since -1000 < any real logit). The host adds
back sum(exp(attack logits)) to reconstruct the full Z.

Per-core device pipeline (4096 rows = 8 supertiles of [128 part x 4 rows x
1000 cols] fp16; 4 consecutive DRAM rows per partition = 8 KB contiguous
DMA descriptors):
  - max: TENSOR_TENSOR max fold tree at DVE 2x rate (1000->500->250->125
    cols, all-16-bit operands hit the 2x perf mode; TENSOR_REDUCE is
    always 1x on this runtime) + one final 1x TENSOR_REDUCE of 125 cols.
  - Z: split to balance engines. For ACT_Z supertiles the ACT engine
    accumulates the row sum during the exp pass (per-group instructions,
    output to a broadcast dummy); for the rest, ACT does one wide pure exp
    and the DVE sums E with the same 2x fold tree + 1x TR(add, f32 out).
fp16 (not bf16): logits are N(0,1) so range is irrelevant and the 3 extra
mantissa bits put input-quantization error at ~7e-4 norm-rel.
"""

import numpy as np

import concourse.bacc as bacc
import concourse.bass as bass  # noqa: F401
import concourse.mybir as mybir
import concourse.tile as tile
from concourse.bass_utils import run_bass_kernel_spmd

B, C, K = 32768, 1000, 10
N_CORES = 8
ROWS = B // N_CORES  # 4096 rows per core
P = 128  # SBUF partitions
R = 4  # consecutive DRAM rows packed per partition
NS = ROWS // (P * R)  # 8 supertiles per core
NT = NS * R  # 32 output columns per partition
N_ACT_Z = 4  # last N supertiles accumulate Z on the ACT engine
MASK_VAL = np.float16(-1000.0)
CC = 5.0

f16 = mybir.dt.float16
f32 = mybir.dt.float32

_CACHE = {}


def build_nc(rows=ROWS):
    """Build the Bass program for one core's shard of `rows` rows."""
    assert rows == NS * P * R

    nc = bacc.Bacc("TRN2", target_bir_lowering=False, debug=False)

    yp = nc.dram_tensor("yp", [rows, C], f16, kind="ExternalInput").ap()
    mx_out = nc.dram_tensor("mx", [P, NT], f16, kind="ExternalOutput").ap()
    zs_out = nc.dram_tensor("zs", [P, NT], f32, kind="ExternalOutput").ap()

    # supertile u, partition p holds DRAM rows u*512 + p*4 + [0..3]
    ypt = yp.rearrange("(u p r) c -> u p (r c)", p=P, r=R)

    Alu = mybir.AluOpType
    Act = mybir.ActivationFunctionType
    X = mybir.AxisListType.X

    with tile.TileContext(nc) as tc:
        with (
            tc.tile_pool(name="singles", bufs=1) as singles,
            tc.tile_pool(name="lg", bufs=5) as lgp,
            tc.tile_pool(name="ep", bufs=4) as epp,
            tc.tile_pool(name="scr", bufs=3) as scrp,
        ):
            MX = singles.tile([P, NT], f16)
            ZS = singles.tile([P, NT], f32)

            def fold_tree(src3, rr, op, tag):
                """[P, rr, 1000] -> [P, rr, 125] via three 2x TT folds."""
                f1 = scrp.tile([P, R * 500], f16, tag=tag + "1")
                f13 = f1[:, 0 : rr * 500].rearrange("p (r c) -> p r c", c=500)
                nc.vector.tensor_tensor(
                    out=f13, in0=src3[:, :, 0:500], in1=src3[:, :, 500:1000], op=op
                )
                f2 = scrp.tile([P, R * 250], f16, tag=tag + "2")
                f23 = f2[:, 0 : rr * 250].rearrange("p (r c) -> p r c", c=250)
                nc.vector.tensor_tensor(
                    out=f23, in0=f13[:, :, 0:250], in1=f13[:, :, 250:500], op=op
                )
                f3 = scrp.tile([P, R * 125], f16, tag=tag + "3")
                f33 = f3[:, 0 : rr * 125].rearrange("p (r c) -> p r c", c=125)
                nc.vector.tensor_tensor(
                    out=f33, in0=f23[:, :, 0:125], in1=f23[:, :, 125:250], op=op
                )
                return f33

            def emit_compute(lg, u, r0, rr, act_z):
                """Process rows [r0, r0+rr) of supertile u (tile `lg`)."""
                lg3 = lg[:, r0 * C : (r0 + rr) * C].rearrange(
                    "p (r c) -> p r c", c=C
                )
                t0 = u * R + r0
                # complement max of raw logits: 2x fold tree + 1x final TR
                m33 = fold_tree(lg3, rr, Alu.max, "m")
                nc.vector.tensor_reduce(
                    out=MX[:, t0 : t0 + rr], in_=m33, axis=X, op=Alu.max
                )
                if act_z:
                    # Z on the ACT accumulator (per-group exp, dummy output)
                    for r in range(rr):
                        dummy = scrp.tile([P, 1], f32, tag="dm")
                        nc.scalar.activation(
                            out=dummy[:].broadcast_to([P, C]),
                            in_=lg3[:, r, :],
                            func=Act.Exp,
                            accum_out=ZS[:, t0 + r : t0 + r + 1],
                        )
                else:
                    # Z on DVE: wide pure exp, then 2x fold tree + 1x TR add
                    E = epp.tile([P, R * C], f16)
                    nc.scalar.activation(
                        out=E[:, 0 : rr * C], in_=lg[:, r0 * C : (r0 + rr) * C],
                        func=Act.Exp,
                    )
                    E3 = E[:, 0 : rr * C].rearrange("p (r c) -> p r c", c=C)
                    s33 = fold_tree(E3, rr, Alu.add, "s")
                    nc.vector.tensor_reduce(
                        out=ZS[:, t0 : t0 + rr], in_=s33, axis=X, op=Alu.add
                    )

            # first supertiles arrive in row-chunks so compute starts as
            # soon as the first 256 KB lands and isn't gated on the DMA ramp
            CHUNKS = {0: (1, 1, 2), 1: (2, 2)}
            for u in range(NS):
                lg = lgp.tile([P, R * C], f16)
                act_z = u >= NS - N_ACT_Z
                r0 = 0
                for rr in CHUNKS.get(u, (R,)):
                    nc.sync.dma_start(
                        out=lg[:, r0 * C : (r0 + rr) * C],
                        in_=ypt[u, :, r0 * C : (r0 + rr) * C],
                    )
                    emit_compute(lg, u, r0, rr, act_z)
                    r0 += rr

            # bulk of mx/zs is ready after supertile 6; issue those on the
            # (now idle) Sync queue, and the last supertile's slices from the
            # Vector/Scalar queues so each trigger fires right after its
            # producer without serializing on one queue.
            tl = (NS - 1) * R
            nc.sync.dma_start(out=mx_out[:, 0:tl], in_=MX[:, 0:tl])
            nc.sync.dma_start(out=zs_out[:, 0:tl], in_=ZS[:, 0:tl])
            nc.sync.dma_start(out=mx_out[:, tl:NT], in_=MX[:, tl:NT])
            nc.scalar.dma_start(out=zs_out[:, tl:NT], in_=ZS[:, tl:NT])

    nc.compile()
    return nc


def _uncol(a):
    """[P, NT] per-core output -> [ROWS] vector; row u*512 + p*4 + r."""
    return np.ascontiguousarray(
        a.reshape(P, NS, R).transpose(1, 0, 2).reshape(ROWS)
    )


def kernel(y_pred, y_attack, _trace=False, _trace_kwargs=None):
    """Full-input entry point: shards across 8 NeuronCores, returns [B] f32."""
    y_pred = np.asarray(y_pred, dtype=np.float32)
    y_attack = np.asarray(y_attack, dtype=np.int32)
    assert y_pred.shape == (B, C) and y_attack.shape == (B, K)

    if "nc" not in _CACHE:
        _CACHE["nc"] = build_nc(ROWS)
    nc = _CACHE["nc"]

    ya = y_attack.astype(np.int64)
    rows_idx = np.arange(B)[:, None]
    yh = y_pred.astype(np.float16)
    attl = np.take_along_axis(yh, ya, axis=1).astype(np.float64)  # [B, K]
    yh[rows_idx, ya] = MASK_VAL

    in_maps = [
        {"yp": np.ascontiguousarray(yh[c * ROWS : (c + 1) * ROWS])}
        for c in range(N_CORES)
    ]
    kwargs = dict(_trace_kwargs or {})
    res = run_bass_kernel_spmd(
        nc, in_maps, core_ids=list(range(N_CORES)), trace=_trace, **kwargs
    )

    MXv = np.empty((B,), dtype=np.float64)
    ZC = np.empty((B,), dtype=np.float64)
    for c in range(N_CORES):
        s = slice(c * ROWS, (c + 1) * ROWS)
        MXv[s] = _uncol(np.asarray(res.results[c]["mx"], dtype=np.float64))
        ZC[s] = _uncol(np.asarray(res.results[c]["zs"], dtype=np.float64))

    # Host epilogue in f64 (O(B*K) work), mirroring reference's f32 casts
    # after each generalized mean.
    att_exp = np.exp(attl)  # [B, K]
    Z = ZC + att_exp.sum(axis=1)
    p_att = att_exp / Z[:, None]
    cmax = np.exp(MXv) / Z
    macro = cmax - p_att.min(axis=1)
    s9 = CC + CC * np.diff(p_att, axis=1)
    gm9 = (np.mean(s9**9.0, axis=1)) ** (1.0 / 9.0)
    gm9 = gm9.astype(np.float32).astype(np.float64)
    sorting = (gm9 - CC) / CC
    catted = np.stack([macro, sorting], axis=-1)
    gm10 = (np.mean((CC + CC * catted) ** 10.0, axis=1)) ** 0.1
    y = ((gm10.astype(np.float32) - CC) / CC).astype(np.float32)

    if _trace:
        return y, res
    return y
